# revision 30
# baseline (speedup 1.0000x reference)
"""Trainium2 Bass kernel for a binarized 4-layer MLP (dense_mlp).

Net (per reference):
  h = sign(x) @ sign(w1).T + b1 ; h = clip(bn1(h), -1, 1)
  h = sign(h) @ sign(w2).T + b2 ; h = clip(bn2(h), -1, 1)
  h = sign(h) @ sign(w3).T + b3 ; h = clip(bn3(h), -1, 1)
  logits = h @ w4.T + b4 ; out = log_softmax(logits)   # 2 classes

Strategy: pure data parallel over 8 cores (batch 131072 -> 8 x 16384).
Host prep: x cast to bf16, transposed to feature-major [81, B], sharded by
columns; sign/BN/bias/log-softmax algebra folded into device weights +
per-feature thresholds/bounds.

On-device per core (feature-major activations, exact binarized matmuls):
  - u0 = [x > 0] in {0,1} bf16 (GpSimd is_gt, keeps DVE free)
  - sign activations propagate as {0,1} "u-form" (DVE is_gt) or +-1 "s-form"
    (ACT Sign); the 2x / rowsum corrections fold into the next layer's
    weights (+-2, exact in fp8) and per-feature thresholds (host).
  - L3: since s3 > 0, clip(s3*p3+e3) = s3*clamp(p3, lo_f, hi_f)+e3 with
    lo=(-1-e3)/s3, hi=(1-e3)/s3: ONE DVE tensor_scalar (min,max) per chunk;
    the affine (s3, e3) folds into the head weights dw2 = dw*s3 and bias
    db2 = db + sum(dw*e3).
  - head: d' = dw2 @ c + db2 accumulated in [1,512] PSUM tiles; GpSimd
    copies psum->SBUF fusing +db2; DRAM bounce re-spreads to batch-major
    [32,128]; then S = softplus(d') on ACT and
    out1 = logsigmoid(d') = d' - S, out0 = logsigmoid(-d') = -S on DVE.
"""

import os
import sys

import numpy as np

for _p in ("/opt/trn_rl_repo", "/root/.axon_site/_ro/trn_rl_repo"):
    if os.path.isdir(_p) and _p not in sys.path:
        sys.path.insert(0, _p)

import ml_dtypes  # noqa: E402

BF16 = ml_dtypes.bfloat16
FP8 = ml_dtypes.float8_e4m3

# Problem constants (hardcoded per contract)
B_FULL = 131072
N_CORES = 8
NB = B_FULL // N_CORES  # 16384 rows per core
IN = 81
H = 384
EPS = 1e-5
P = 128
WCH = 1024          # free-dim per elementwise tile (2 PSUM banks)
G_NCH = 8           # 512-col chunks per super-chunk
G_COLS = G_NCH * 512  # 4096
N_GROUPS = NB // G_COLS
WPG = G_COLS // WCH
RPT = G_COLS // P  # tail rows per group

_CACHE = {}


def _build_program():
    import concourse.bacc as bacc
    import concourse.bass as bass  # noqa: F401
    import concourse.tile as tile
    from concourse import mybir
    from concourse.hw_specs import get_activation_tables

    f32 = mybir.dt.float32
    bf16 = mybir.dt.bfloat16
    fp16 = mybir.dt.float16
    fp8 = mybir.dt.float8e4
    DR = mybir.MatmulPerfMode.DoubleRow
    AF = mybir.ActivationFunctionType
    ALU = mybir.AluOpType

    nc = bacc.Bacc("TRN2", target_bir_lowering=False, debug=False)

    xt_d = nc.dram_tensor("xt", [IN, NB], bf16, kind="ExternalInput").ap()
    w1t_d = nc.dram_tensor("w1t", [IN, H], bf16, kind="ExternalInput").ap()
    w2t_d = nc.dram_tensor("w2t", [P, 1536], fp8, kind="ExternalInput").ap()
    w3t_d = nc.dram_tensor("w3t", [P, 1536], fp8, kind="ExternalInput").ap()
    dwt_d = nc.dram_tensor("dwt", [P, 3], fp16, kind="ExternalInput").ap()
    aux_d = nc.dram_tensor("aux", [P, 13], f32, kind="ExternalInput").ap()
    out_d = nc.dram_tensor("out", [NB, 2], f32, kind="ExternalOutput").ap()

    with tile.TileContext(nc) as tc:
        with (
            tc.tile_pool(name="consts", bufs=1) as cpool,
            tc.tile_pool(name="xin", bufs=4) as xpool,
            tc.tile_pool(name="u0", bufs=8) as u0pool,
            tc.tile_pool(name="acts", bufs=8) as apool,
            tc.tile_pool(name="h3", bufs=14) as h3pool,
            tc.tile_pool(name="dsb", bufs=1) as dsbpool,
            tc.tile_pool(name="fin", bufs=2) as fpool,
            tc.tile_pool(name="mm", bufs=3, space="PSUM") as pspool,
            tc.tile_pool(name="mmd", bufs=2, space="PSUM") as psdpool,
            tc.tile_pool(name="dram", bufs=1, space="DRAM") as dpool,
        ):
            dscr = dpool.tile([NB], f32)

            # Load the one activation table that serves every function this
            # kernel uses (Sign, Identity, Exp, Ln) so no further table
            # loads are ever inserted (greedy insertion would otherwise
            # thrash exp<->ln, costing 1.3us in the output tail).
            try:
                tnames = list(get_activation_tables("TRN2"))
                tbl = tnames.index("natural_log_exp_and_others")
                nc.scalar.add_instruction(
                    mybir.InstLoadActFuncSet(
                        act_func_set_id=tbl,
                        name=nc.get_next_instruction_name(),
                        ins=[],
                        outs=[],
                    )
                )
            except Exception:
                pass  # fall back to automatic insertion

            # xf load + Sign, one group's worth; [81, WCH] u0 tiles so the
            # first matmul can start early and L1 consumes tile-by-tile.
            def emit_u0(g):
                col0 = g * G_COLS
                tiles = []
                # group 0: single-WCH DMA pieces so the first Sign (and
                # first matmul) starts as early as possible
                npc = 1 if g == 0 else 2
                for w in range(WPG // npc):
                    xf = xpool.tile(
                        [IN, npc * WCH], bf16, tag="xf", name=f"xf_{g}_{w}"
                    )
                    nc.sync.dma_start(
                        xf[:],
                        xt_d[:, col0 + w * npc * WCH : col0 + (w + 1) * npc * WCH],
                    )
                    for hw_ in range(npc):
                        u0 = u0pool.tile(
                            [IN, WCH], bf16, tag="u0", name=f"u0_{g}_{w}_{hw_}"
                        )
                        nc.scalar.activation(
                            u0[:], xf[:, hw_ * WCH : (hw_ + 1) * WCH], AF.Sign
                        )
                        tiles.append(u0)
                return tiles

            # group-0 input DMA issued before the (larger) const DMAs
            u0_cur = emit_u0(0)

            # ---- constants ----
            w1s = cpool.tile([IN, H], bf16)
            nc.sync.dma_start(w1s[:], w1t_d[:])
            aux = cpool.tile([P, 13], f32)
            nc.sync.dma_start(aux[:], aux_d[:])
            w2s = cpool.tile([P, 1536], fp8)
            nc.sync.dma_start(w2s[:], w2t_d[:])
            w3s = cpool.tile([P, 1536], fp8)
            nc.sync.dma_start(w3s[:], w3t_d[:])
            w2r = w2s.rearrange("p (s i c) -> p s i c", i=2, c=P)
            w3r = w3s.rearrange("p (s i c) -> p s i c", i=2, c=P)
            dws = cpool.tile([P, 3], fp16)
            nc.sync.dma_start(dws[:], dwt_d[:])

            # persistent head tiles: d2all[p, 128g+j] = d'[batch 4096g+128p+j]
            d2all = cpool.tile([RPT, 512], f32)
            exall = cpool.tile([RPT, 512], f32)

            for g in range(N_GROUPS):
                u0t = u0_cur

                # ---- L1 ----
                u1 = []
                for w in range(WPG):
                    ua = apool.tile([P, 3, WCH], fp8, tag="u1")
                    u1.append(ua)
                for m in range(3):
                    for w in range(WPG):
                        ps = pspool.tile([P, WCH], f32, tag="ps")
                        for h in range(2):
                            nc.tensor.matmul(
                                ps[:, h * 512 : (h + 1) * 512],
                                w1s[:, m * P : (m + 1) * P],
                                u0t[w][:, h * 512 : (h + 1) * 512],
                                start=True,
                                stop=True,
                            )
                        if m < 2:
                            nc.scalar.activation(
                                u1[w][:, m, :], ps[:], AF.Sign,
                                bias=aux[:, m : m + 1], scale=1.0
                            )
                        else:
                            nc.vector.tensor_scalar(
                                u1[w][:, m, :], ps[:], aux[:, 2:3], None,
                                ALU.is_gt
                            )

                # ---- L2 (fp8 DoubleRow, K=512 padded) ----
                u2 = []
                for w in range(WPG):
                    ua = apool.tile([P, 3, WCH], fp8, tag="u2")
                    u2.append(ua)
                for m in range(3):
                    for wp in range(WPG // 2):
                        pss = [
                            pspool.tile([P, WCH], f32, tag="ps", name=f"ps2_{g}_{m}_{wp}_{wi}")
                            for wi in range(2)
                        ]
                        # kh-outer, 4 MMs per weight load
                        for kh in range(2):
                            for wi in range(2):
                                w = wp * 2 + wi
                                for h in range(2):
                                    nc.tensor.matmul(
                                        pss[wi][:, h * 512 : (h + 1) * 512],
                                        w2r[:, kh * 3 + m, :, :],
                                        u1[w][:, kh : kh + 2,
                                              h * 512 : (h + 1) * 512],
                                        start=(kh == 0),
                                        stop=(kh == 1),
                                        perf_mode=DR,
                                    )
                        for wi in range(2):
                            w = wp * 2 + wi
                            if m < 2:
                                nc.scalar.activation(
                                    u2[w][:, m, :], pss[wi][:], AF.Sign,
                                    bias=aux[:, 3 + m : 4 + m], scale=1.0
                                )
                            else:
                                nc.vector.tensor_scalar(
                                    u2[w][:, m, :], pss[wi][:], aux[:, 5:6],
                                    None, ALU.is_gt
                                )

                # ---- L3 (fp8 DoubleRow) + clamp (bounds absorb BN affine) ----
                h3 = [[None] * WPG for _ in range(3)]
                for m in range(3):
                    for wp in range(WPG // 2):
                        pss = [
                            pspool.tile([P, WCH], f32, tag="ps", name=f"ps3_{g}_{m}_{wp}_{wi}")
                            for wi in range(2)
                        ]
                        for kh in range(2):
                            for wi in range(2):
                                w = wp * 2 + wi
                                for h in range(2):
                                    nc.tensor.matmul(
                                        pss[wi][:, h * 512 : (h + 1) * 512],
                                        w3r[:, kh * 3 + m, :, :],
                                        u2[w][:, kh : kh + 2,
                                              h * 512 : (h + 1) * 512],
                                        start=(kh == 0),
                                        stop=(kh == 1),
                                        perf_mode=DR,
                                    )
                        for wi in range(2):
                            w = wp * 2 + wi
                            # c = clamp(p3, lo_f, hi_f) -> fp16 (one DVE op;
                            # fp16 keeps integer p3 and fp16-rounded bounds exact)
                            h3c = h3pool.tile([P, WCH], fp16, tag="h3", name=f"h3_{g}_{m}_{wp}_{wi}")
                            nc.vector.tensor_scalar(
                                h3c[:], pss[wi][:],
                                aux[:, 6 + m : 7 + m], aux[:, 9 + m : 10 + m],
                                ALU.min, ALU.max,
                            )
                            h3[m][w] = h3c

                # prefetch next group's input compare ahead of the head's
                # ACT work, so L1(g+1) matmuls never wait on the ACT queue
                if g + 1 < N_GROUPS:
                    u0_cur = emit_u0(g + 1)

                # ---- head: d' per 512-chunk in [1,512] PSUM tiles ----
                dsb = dsbpool.tile([1, G_COLS], f32)
                for r in range(G_NCH):
                    w, h = r // 2, r % 2
                    psd = psdpool.tile([1, 512], f32, tag="psd")
                    for k in range(3):
                        nc.tensor.matmul(
                            psd[:],
                            dws[:, k : k + 1],
                            h3[k][w][:, h * 512 : (h + 1) * 512],
                            start=(k == 0),
                            stop=(k == 2),
                        )
                    # psum -> SBUF on ACT (Identity), fusing +db2
                    nc.scalar.activation(
                        dsb[0:1, r * 512 : (r + 1) * 512], psd[:],
                        AF.Identity, bias=aux[0:1, 12:13],
                    )

                # re-spread to batch-major via DRAM bounce
                # (direct SBUF->SBUF partition-spread DMA scrambles on HW)
                dsl = dscr[g * G_COLS : (g + 1) * G_COLS]
                nc.sync.dma_start(
                    dsl.rearrange("(one f) -> one f", one=1), dsb[:]
                )
                # gather so d2all[p, 128g+j] = d'[batch 4096g+128p+j]
                # (contiguous 512B per partition -> efficient DMA)
                nc.sync.dma_start(
                    d2all[:, g * P : (g + 1) * P],
                    dsl.rearrange("(p j) -> p j", j=P),
                )
                # E = exp(-d') per group (exp_and_others table serves Sign,
                # Identity and Exp -> no table swaps until the final Ln)
                nc.scalar.activation(
                    exall[:, g * P : (g + 1) * P],
                    d2all[:, g * P : (g + 1) * P], AF.Exp, scale=-1.0,
                )

            # ---- final: S1 = ln(E+1) = softplus(-d') for all groups ----
            # out1 = logsigmoid(d') = -S1 ; out0 = logsigmoid(-d') = -d'-S1
            sp = fpool.tile([RPT, 512], f32, tag="sp")
            nc.scalar.activation(sp[:], exall[:], AF.Ln, bias=1.0)
            out_t = fpool.tile([RPT, 2 * 512], f32, tag="outt")
            ov = out_t.rearrange("p (j c) -> p j c", c=2)
            nc.vector.tensor_scalar(ov[:, :, 1], sp[:], -1.0, None, ALU.mult)
            nc.vector.scalar_tensor_tensor(
                ov[:, :, 0], d2all[:], -1.0, sp[:], ALU.mult, ALU.subtract
            )
            nc.sync.dma_start(
                out_d.rearrange("(g p j) c -> p g j c", g=N_GROUPS, j=P),
                out_t.rearrange("p (g j c) -> p g j c", g=N_GROUPS, c=2),
            )

    nc.compile()
    return nc


def _get_program():
    if "nc" not in _CACHE:
        _CACHE["nc"] = _build_program()
    return _CACHE["nc"]


def _prep_consts(w1, b1, w2, b2, w3, b3, w4, b4,
                 g1, be1, m1, v1, g2, be2, m2, v2, g3, be3, m3, v3):
    """Host-side folding. Returns dict of device const arrays."""
    f8 = np.float64
    w1 = np.asarray(w1, f8); w2 = np.asarray(w2, f8); w3 = np.asarray(w3, f8)
    w4 = np.asarray(w4, f8)
    b1 = np.asarray(b1, f8); b2 = np.asarray(b2, f8); b3 = np.asarray(b3, f8)
    b4 = np.asarray(b4, f8)

    def fold(g, be, m, v, b):
        s = np.asarray(g, f8) / np.sqrt(np.asarray(v, f8) + EPS)
        c = s * (b - np.asarray(m, f8)) + np.asarray(be, f8)
        return s, c

    s1, c1 = fold(g1, be1, m1, v1, b1)
    s2, c2 = fold(g2, be2, m2, v2, b2)
    s3, c3 = fold(g3, be3, m3, v3, b3)

    W1s = np.sign(w1)  # [384, 81]
    W2s = np.sign(w2)  # [384, 384]
    W3s = np.sign(w3)

    # L1: input activations are s-form (+-1 from ACT Sign) -> weights +-1
    w1t = W1s.T.astype(BF16)  # [81, 384]

    # L2/L3 inputs: m0/m1 chunks (f<256) s-form (+-1), m2 u-form (x2)
    multf = np.where(np.arange(H) < 2 * P, 1.0, 2.0)
    W2eff = W2s * multf[None, :]
    W3eff = W3s * multf[None, :]

    def pack_lhsT_dr(Weff):
        # DoubleRow packing with overlapping rhs windows: kh=0 reads
        # activation planes (0,1) = features 0..255; kh=1 reads planes
        # (1,2) = features 128..383 with ZERO weights on the repeated
        # plane 1 (i=0), so no pad plane / memset is needed.
        t = np.zeros((P, 6, 2, P), dtype=f8)
        for m in range(3):
            for i in range(2):  # kh=0: features i*128..
                blk = Weff[m * P : (m + 1) * P, i * P : (i + 1) * P]
                t[:, m, i, :] = blk.T
            # kh=1: i=0 stays zero; i=1 = features 256..383
            blk = Weff[m * P : (m + 1) * P, 2 * P : 3 * P]
            t[:, 3 + m, 1, :] = blk.T
        return t.reshape(P, 1536).astype(FP8)

    w2t = pack_lhsT_dr(W2eff)
    w3t = pack_lhsT_dr(W3eff)

    # thresholds: u = [p > phi];  s-form ACT: sign(p - phi)
    phi1 = -c1 / s1  # s-form u0: no rowsum correction
    phi2 = W2s[:, 2 * P :].sum(axis=1) - c2 / s2
    # L3: y3 = s3*p3 + e3; clip bounds on raw p3 (s3 > 0). Bounds rounded
    # to fp16 on the host so device-side clamp outputs are fp16-exact
    # (interior p3 values are small integers, exact in fp16).
    e3 = c3 - s3 * W3s[:, 2 * P :].sum(axis=1)
    hi3 = ((1.0 - e3) / s3).astype(np.float16).astype(f8)
    lo3 = ((-1.0 - e3) / s3).astype(np.float16).astype(f8)

    dw = w4[1] - w4[0]
    db = b4[1] - b4[0]
    dw2 = dw * s3                     # head weights absorb BN scale
    dw2r = dw2.astype(np.float16).astype(f8)
    # db2 absorbs BN shift and the clamp-window-center component of the
    # fp16 head-weight rounding error (cuts cancellation amplification).
    mid3 = 0.5 * (hi3 + lo3)
    db2 = db + float((dw * e3).sum()) + float(((dw2 - dw2r) * mid3).sum())

    dwt = np.zeros((P, 3), dtype=f8)
    for k in range(3):
        dwt[:, k] = dw2[k * P : (k + 1) * P]
    dwt = dwt.astype(np.float16)

    aux = np.zeros((P, 13), dtype=f8)
    aux[:, 0] = -phi1[0:P]
    aux[:, 1] = -phi1[P : 2 * P]
    aux[:, 2] = phi1[2 * P : 3 * P]
    aux[:, 3] = -phi2[0:P]
    aux[:, 4] = -phi2[P : 2 * P]
    aux[:, 5] = phi2[2 * P : 3 * P]
    for m in range(3):
        aux[:, 6 + m] = hi3[m * P : (m + 1) * P]
        aux[:, 9 + m] = lo3[m * P : (m + 1) * P]
    aux[:, 12] = db2
    aux = aux.astype(np.float32)

    return {"w1t": w1t, "w2t": w2t, "w3t": w3t, "dwt": dwt, "aux": aux}


def _make_in_maps(inputs):
    x = np.asarray(inputs["x"], np.float32)
    xt = np.ascontiguousarray(x.T.astype(BF16))  # [81, 131072] feature-major
    consts = _prep_consts(
        inputs["w1"], inputs["b1"], inputs["w2"], inputs["b2"],
        inputs["w3"], inputs["b3"], inputs["w4"], inputs["b4"],
        inputs["g1"], inputs["be1"], inputs["m1"], inputs["v1"],
        inputs["g2"], inputs["be2"], inputs["m2"], inputs["v2"],
        inputs["g3"], inputs["be3"], inputs["m3"], inputs["v3"],
    )
    in_maps = []
    for i in range(N_CORES):
        m = {"xt": np.ascontiguousarray(xt[:, i * NB : (i + 1) * NB])}
        m.update(consts)
        in_maps.append(m)
    return in_maps


def kernel(**inputs):
    from concourse.bass_utils import run_bass_kernel_spmd

    nc = _get_program()
    in_maps = _make_in_maps(inputs)
    res = run_bass_kernel_spmd(nc, in_maps, list(range(N_CORES)))
    out = np.concatenate([res.results[i]["out"] for i in range(N_CORES)], axis=0)
    return out.astype(np.float32)
